# revision 1
# baseline (speedup 1.0000x reference)
"""Trainium2 Bass kernel for DilatedCausalSelfAttention (B=1, L=4096, E=1024,
16 heads, d=64; branches (w,r) = (1024,1), (2048,2), (4096,4)).

Distribution: head-sharded, 2 heads per core (core c owns heads 2c, 2c+1).
Each core computes q/k/v for its heads directly in per-branch sparse coords
(host pre-gathers x^T columns per branch so the SPMD program is uniform),
runs windowed causal attention per branch entirely in SBUF, combines branches
with 1/V(p,h) weights (the reference's probs-LSE softmax weights equal 1/V to
~2e-4 because lse_k = log(g + delta_k), delta in [1, 1.72], g = 1024), then a
single AllToAll redistributes attn^T so every core projects a disjoint block
of 512 sequence rows. Host-side work is only slicing/transpose/concat.
"""

import numpy as np

import concourse.bacc as bacc
import concourse.tile as tile
from concourse import mybir
from concourse.bass_utils import run_bass_kernel_spmd

F32 = mybir.dt.float32
F32R = mybir.dt.float32r
BF16 = mybir.dt.bfloat16

N_CORES = 8
L = 4096
E = 1024
D = 64
G = 1024                      # sparse window length (w // r, same for all branches)
KT = 8                        # 128-row key tiles per window
RATIOS = [1, 2, 4]
LBS = [L // r for r in RATIOS]          # per-branch sparse length
VOFF = [0, LBS[0], LBS[0] + LBS[1]]     # offsets into concatenated vpat
NEG = -30000.0


def build_nc():
    nc = bacc.Bacc("TRN2", target_bir_lowering=False, debug=False,
                   num_devices=N_CORES)

    xts = [nc.dram_tensor(f"xt{b}", [E, LBS[b]], BF16, kind="ExternalInput").ap()
           for b in range(3)]
    wq = nc.dram_tensor("wq", [E, 128], BF16, kind="ExternalInput").ap()
    wk = nc.dram_tensor("wk", [E, 128], BF16, kind="ExternalInput").ap()
    wv = nc.dram_tensor("wv", [E, 128], BF16, kind="ExternalInput").ap()
    wproj = nc.dram_tensor("wproj", [E, E], F32R, kind="ExternalInput").ap()
    ident = nc.dram_tensor("ident", [128, 128], F32R, kind="ExternalInput").ap()
    trimask = nc.dram_tensor("trimask", [128, 128], BF16, kind="ExternalInput").ap()
    vpat = nc.dram_tensor("vpat", [1, sum(LBS)], F32, kind="ExternalInput").ap()
    out = nc.dram_tensor("out", [512, E], F32, kind="ExternalOutput").ap()

    from contextlib import ExitStack
    with tile.TileContext(nc) as tc, ExitStack() as stk:
        # ---- persistent pools -------------------------------------------------
        consts = stk.enter_context(tc.tile_pool(name="consts", bufs=1))
        ident_sb = consts.tile([128, 128], F32R)
        nc.sync.dma_start(ident_sb[:], ident[:])
        tri_sb = consts.tile([128, 128], BF16)
        nc.sync.dma_start(tri_sb[:], trimask[:])
        vpat_sb = consts.tile([1, sum(LBS)], F32)
        nc.sync.dma_start(vpat_sb[:], vpat[:])
        w_sb = {}
        for name, ap in (("q", wq), ("k", wk), ("v", wv)):
            t = consts.tile([128, 8 * 128], BF16, name=f"w{name}sb")
            for k in range(8):
                nc.sync.dma_start(t[:, 128 * k:128 * (k + 1)],
                                  ap[128 * k:128 * (k + 1), :])
            w_sb[name] = t

        ftp = stk.enter_context(tc.tile_pool(name="ftp", bufs=1))
        FT = [ftp.tile([128, LBS[b]], BF16, name=f"FT{b}") for b in range(3)]
        dram = stk.enter_context(tc.tile_pool(name="dram", bufs=1, space="DRAM"))
        qkt = stk.enter_context(tc.tile_pool(name="qkt", bufs=1))
        QT = [qkt.tile([128, LBS[b]], F32R, name=f"QT{b}") for b in range(3)]
        KTb = [qkt.tile([128, LBS[b]], F32R, name=f"KT{b}") for b in range(3)]
        vaugp = stk.enter_context(tc.tile_pool(name="vaugp", bufs=1))
        # V_aug per branch: tile t block of 130 cols = [h0 V|1][h1 V|1]
        Vaug = [vaugp.tile([128, (LBS[b] // 128) * 130], BF16, name=f"Va{b}")
                for b in range(3)]

        for b in range(3):
            ones3 = Vaug[b][:].rearrange("p (t c) -> p t c", c=65)
            nc.vector.memset(ones3[:, :, 64:65], 1.0)

        # ---- P1: per-branch QKV (+ V transpose into V_aug) --------------------
        with (tc.tile_pool(name="xtp", bufs=1) as xtp,
              tc.tile_pool(name="vtp", bufs=1) as vtp,
              tc.tile_pool(name="qkvps", bufs=2, space="PSUM") as qkvps,
              tc.tile_pool(name="trps", bufs=2, space="PSUM") as trps):
            VT = [vtp.tile([128, LBS[b]], F32R, name=f"VT{b}") for b in range(3)]
            for b in range(3):
                nblk = LBS[b] // 512
                for s_ in range(nblk):
                    xtiles = []
                    for k in range(8):
                        xt_t = xtp.tile([128, 512], BF16, tag=f"xt{k}", bufs=2,
                                        name=f"xt_t{k}")
                        nc.sync.dma_start(
                            xt_t[:], xts[b][128 * k:128 * (k + 1),
                                            512 * s_:512 * (s_ + 1)])
                        xtiles.append(xt_t)
                    for nm, dst in (("q", QT[b]), ("k", KTb[b]), ("v", VT[b])):
                        ps = qkvps.tile([128, 512], F32, tag=f"ps{nm}",
                                        name=f"ps{nm}")
                        for k in range(8):
                            nc.tensor.matmul(ps[:],
                                             w_sb[nm][:, 128 * k:128 * (k + 1)],
                                             xtiles[k][:], start=(k == 0),
                                             stop=(k == 7))
                        nc.vector.tensor_copy(dst[:, 512 * s_:512 * (s_ + 1)],
                                              ps[:])
                for t in range(LBS[b] // 128):
                    ptr = trps.tile([128, 128], F32, tag="tr", name="ptr")
                    nc.tensor.transpose(ptr[:].bitcast(F32R),
                                        VT[b][:, 128 * t:128 * (t + 1)],
                                        ident_sb[:])
                    vdst = Vaug[b][:, 130 * t:130 * (t + 1)]
                    nc.vector.tensor_copy(
                        vdst.rearrange("p (h c) -> p h c", c=65)[:, :, 0:64],
                        ptr[:].rearrange("p (h c) -> p h c", c=64))

        # ---- P2: attention ----------------------------------------------------
        with (tc.tile_pool(name="spps", bufs=2, space="PSUM") as spps,
              tc.tile_pool(name="ops", bufs=1, space="PSUM") as ops,
              tc.tile_pool(name="esp", bufs=3) as esp,
              tc.tile_pool(name="smallp", bufs=2) as smallp,
              tc.tile_pool(name="bcp", bufs=2) as bcp):
            for b in (1, 2, 0):
                nwin = LBS[b] // G
                for n in range(nwin):
                    O = [ops.tile([65, G], F32, tag=f"o{hh}", name=f"O{hh}")
                         for hh in range(2)]
                    for kt in range(KT):
                        nq = G - 128 * kt
                        base = G * n + 128 * kt
                        for hh in range(2):
                            hs = 64 * hh
                            sp = spps.tile([128, G], F32, tag="sp", name="sp")
                            lhsT = KTb[b][hs:hs + 64, base:base + 128]
                            if nq > 512:
                                nc.tensor.matmul(sp[:, 0:512], lhsT,
                                                 QT[b][hs:hs + 64, base:base + 512],
                                                 start=True, stop=True)
                                nc.tensor.matmul(sp[:, 512:nq], lhsT,
                                                 QT[b][hs:hs + 64,
                                                       base + 512:G * n + G],
                                                 start=True, stop=True)
                            else:
                                nc.tensor.matmul(sp[:, 0:nq], lhsT,
                                                 QT[b][hs:hs + 64, base:base + nq],
                                                 start=True, stop=True)
                            es = esp.tile([128, G], BF16, tag="es", name="es")
                            nc.scalar.activation(es[:, 0:nq], sp[:, 0:nq],
                                                 mybir.ActivationFunctionType.Exp)
                            nc.vector.tensor_mul(es[:, 0:128], es[:, 0:128],
                                                 tri_sb[:])
                            va = Vaug[b][:, 130 * (KT * n + kt) + 65 * hh:
                                         130 * (KT * n + kt) + 65 * hh + 65]
                            if kt < 4:
                                pv1 = 512 - 128 * kt
                                nc.tensor.matmul(O[hh][:, 128 * kt:512], va,
                                                 es[:, 0:pv1], start=(kt == 0),
                                                 stop=(kt == 3),
                                                 skip_group_check=True)
                                nc.tensor.matmul(O[hh][:, 512:G], va,
                                                 es[:, pv1:nq], start=(kt == 0),
                                                 stop=(kt == 7),
                                                 skip_group_check=True)
                            else:
                                nc.tensor.matmul(O[hh][:, 128 * kt:G], va,
                                                 es[:, 0:nq], start=False,
                                                 stop=(kt == 7),
                                                 skip_group_check=True)
                    for hh in range(2):
                        dstage = smallp.tile([1, G], F32, tag="dstage",
                                             name="dstage")
                        nc.vector.tensor_copy(dstage[:], O[hh][64:65, :])
                        rcp = smallp.tile([1, G], F32, tag="rcp", name="rcp")
                        nc.vector.reciprocal_approx_fast(rcp[:], dstage[:])
                        scl = smallp.tile([1, G], F32, tag="scl", name="scl")
                        nc.vector.tensor_mul(
                            scl[:], rcp[:],
                            vpat_sb[0:1, VOFF[b] + G * n:VOFF[b] + G * (n + 1)])
                        sclb = bcp.tile([64, G], F32, tag="sclb", name="sclb")
                        nc.gpsimd.partition_broadcast(sclb[:], scl[:])
                        nc.vector.tensor_mul(
                            FT[b][64 * hh:64 * hh + 64, G * n:G * (n + 1)],
                            O[hh][0:64, :], sclb[:])

        a2a12_in = dram.tile([1024, 384], BF16)
        a2a12_out = dram.tile([1024, 384], BF16)
        for j in range(8):
            nc.sync.dma_start(a2a12_in[128 * j:128 * (j + 1), 0:256],
                              FT[1][:, 256 * j:256 * (j + 1)])
            nc.sync.dma_start(a2a12_in[128 * j:128 * (j + 1), 256:384],
                              FT[2][:, 128 * j:128 * (j + 1)])
        nc.gpsimd.collective_compute(
            "AllToAll", mybir.AluOpType.bypass,
            replica_groups=[list(range(N_CORES))],
            ins=[a2a12_in.opt()], outs=[a2a12_out.opt()])

        # ---- P3: AllToAll -----------------------------------------------------
        # shard j (128 partitions) = [FT0 512-slice | FT1 256-slice | FT2 128-slice]
        with (tc.tile_pool(name="wpp", bufs=1) as wpp,
              tc.tile_pool(name="ptp", bufs=1) as ptp,
              tc.tile_pool(name="tmpp", bufs=2) as tmpp,
              tc.tile_pool(name="prps", bufs=2, space="PSUM") as prps,
              tc.tile_pool(name="ocp", bufs=2) as ocp):
            wproj_sb = []
            for jj in range(8):
                t = wpp.tile([128, E], F32R, tag=f"wp{jj}")
                nc.sync.dma_start(t[:], wproj[128 * jj:128 * (jj + 1), :])
                wproj_sb.append(t)

            a2a_in = dram.tile([1024, 512], BF16)
            a2a_out = dram.tile([1024, 512], BF16)
            for j in range(8):
                nc.sync.dma_start(a2a_in[128 * j:128 * (j + 1), 0:512],
                                  FT[0][:, 512 * j:512 * (j + 1)])
            nc.gpsimd.collective_compute(
                "AllToAll", mybir.AluOpType.bypass,
                replica_groups=[list(range(N_CORES))],
                ins=[a2a_in.opt()], outs=[a2a_out.opt()])

            # ---- P4: merge branch pieces into dense attn^T block ---------------
            PT = []
            for jj in range(8):
                pt = ptp.tile([128, 512], F32R, tag=f"pt{jj}")
                nc.gpsimd.dma_start(pt[:], a2a_out[128 * jj:128 * (jj + 1), 0:512])
                t1 = tmpp.tile([128, 256], F32R, tag="t1")
                nc.gpsimd.dma_start(t1[:], a2a12_out[128 * jj:128 * (jj + 1), 0:256])
                t2 = tmpp.tile([128, 128], F32R, tag="t2")
                nc.gpsimd.dma_start(t2[:], a2a12_out[128 * jj:128 * (jj + 1), 256:384])
                i2, i4 = jj // 4, jj // 2
                pt2 = pt[:].rearrange("p (t c) -> p t c", c=2)
                nc.vector.tensor_add(pt2[:, :, i2:i2 + 1], pt2[:, :, i2:i2 + 1],
                                     t1[:].rearrange("p (t c) -> p t c", c=1))
                pt4 = pt[:].rearrange("p (t c) -> p t c", c=4)
                nc.vector.tensor_add(pt4[:, :, i4:i4 + 1], pt4[:, :, i4:i4 + 1],
                                     t2[:].rearrange("p (t c) -> p t c", c=1))
                PT.append(pt)

            # ---- P5: projection ------------------------------------------------
            for m in range(4):
                for nb in range(2):
                    pp = prps.tile([128, 512], F32, tag="pp")
                    for jj in range(8):
                        nc.tensor.matmul(pp[:], PT[jj][:, 128 * m:128 * (m + 1)],
                                         wproj_sb[jj][:, 512 * nb:512 * (nb + 1)],
                                         start=(jj == 0), stop=(jj == 7))
                    oc = ocp.tile([128, 512], F32, tag="oc")
                    nc.vector.tensor_copy(oc[:], pp[:])
                    nc.sync.dma_start(out[128 * m:128 * (m + 1),
                                          512 * nb:512 * (nb + 1)], oc[:])
    nc.compile()
    return nc


_NC_CACHE = None


def _get_nc():
    global _NC_CACHE
    if _NC_CACHE is None:
        _NC_CACHE = build_nc()
    return _NC_CACHE


def _host_inputs(x, w_qkv, w_proj):
    xT = np.ascontiguousarray(x[0].T).astype(np.float32)      # (E, L)
    ident = np.eye(128, dtype=np.float32)
    import ml_dtypes
    f = np.arange(128)
    trimask = np.where(f[None, :] >= f[:, None], 1.0, 0.0).astype(ml_dtypes.bfloat16)
    in_maps = []
    for c in range(N_CORES):
        h = 2 * c
        vps = []
        for b, r in enumerate(RATIOS):
            i = h // (16 // r)
            cs = r * np.arange(L // r) + i
            V = 1 + (cs % 2 == h // 8).astype(np.int32) \
                  + (cs % 4 == h // 4).astype(np.int32)
            vps.append((1.0 / V).astype(np.float32))
        i2, i4 = c // 4, c // 2
        m = {
            "xt0": xT,
            "xt1": np.ascontiguousarray(xT[:, i2::2]),
            "xt2": np.ascontiguousarray(xT[:, i4::4]),
            "wq": np.ascontiguousarray(w_qkv[:, 128 * c:128 * (c + 1)]) / 8.0,
            "wk": np.ascontiguousarray(w_qkv[:, E + 128 * c:E + 128 * (c + 1)]),
            "wv": np.ascontiguousarray(w_qkv[:, 2 * E + 128 * c:2 * E + 128 * (c + 1)]),
            "wproj": np.ascontiguousarray(w_proj).astype(np.float32),
            "ident": ident,
            "trimask": trimask,
            "vpat": np.concatenate(vps)[None, :],
        }
        bf = ("trimask", "xt0", "xt1", "xt2", "wq", "wk", "wv")
        in_maps.append({k: np.ascontiguousarray(
                            v if k == "trimask" else
                            np.asarray(v, np.float32).astype(ml_dtypes.bfloat16))
                        if k in bf
                        else np.ascontiguousarray(v, dtype=np.float32)
                        for k, v in m.items()})
    return in_maps


def kernel(x, w_qkv, w_proj, _trace=False):
    x = np.asarray(x, np.float32)
    w_qkv = np.asarray(w_qkv, np.float32)
    w_proj = np.asarray(w_proj, np.float32)
    nc = _get_nc()
    in_maps = _host_inputs(x, w_qkv, w_proj)
    res = run_bass_kernel_spmd(nc, in_maps, core_ids=list(range(N_CORES)),
                               trace=_trace)
    full = np.empty((L, E), np.float32)
    for c in range(N_CORES):
        full[512 * c:512 * (c + 1)] = res.results[c]["out"]
    out = full.reshape(1, L, E)
    if _trace:
        return out, res
    return out



# revision 36
# speedup vs baseline: 235.9472x; 235.9472x over previous
"""Trainium2 Bass kernel for DilatedCausalSelfAttention (B=1, L=4096, E=1024,
16 heads, d=64; branches (w,r) = (1024,1), (2048,2), (4096,4)).

Distribution: head-sharded, 2 heads per core (core c owns heads 2c, 2c+1).
Host pre-gathers x^T columns per branch (phase offsets i2=c//4, i4=c//2 are
per-core data, so gathering must happen host-side for an SPMD program).

Software-pipelined window loop (windows processed b=2, b=1 x2, b=0 x4 so the
branch-1/2 AllToAll overlaps branch-0 compute). Per step: P1 (QKV matmuls for
this window, V transposed into key-major V_aug with a ones column), score
phase (K^T Q -> exp -> es bf16, kept in SBUF), then PV + tail of the
PREVIOUS window — so the PE works on P1/QK matmuls while the Activation
engine chews the previous window's exponentials.

PV uses lhsT=es blocks against rhs=V_aug giving O^T [128 q, 65] PSUM groups
(softmax denominator lands in col 64 as a per-partition scalar; one
accumulation group open per PSUM bank at a time — hardware discards an open
group's pending data if another group starts in the same bank). Tail:
scl = 1/(denom*V) via tensor_scalar, both heads into one [128 q, 128 feat]
bf16 tile per q-tile, DMA-transposed to [feat, q] and staged straight into
the AllToAll DRAM buffers. Two AllToAlls redistribute attn^T; each core then
merges branch pieces (strided adds keyed by source block index) and projects
its 512 rows. All DMAs are batched (HWDGE costs ~625ns per instruction).

The branch combination weights equal 1/V(p,h) (V = #branches covering the
position) to ~2e-4 because the reference's probs-logsumexp weights are
log(g + delta) with g=1024, delta in [1, 1.72].
"""

import numpy as np

import concourse.bacc as bacc
import concourse.tile as tile
from concourse import mybir
from concourse.bass_utils import run_bass_kernel_spmd

F32 = mybir.dt.float32
BF16 = mybir.dt.bfloat16

N_CORES = 8
L = 4096
E = 1024
D = 64
G = 1024
KT = 8
RATIOS = [1, 2, 4]
LBS = [L // r for r in RATIOS]
NWIN = [lb // G for lb in LBS]
TOFF = [0, 32, 48]
STEPS = [(2, 0), (1, 0), (1, 1), (0, 0), (0, 1), (0, 2), (0, 3)]


def build_nc():
    nc = bacc.Bacc("TRN2", target_bir_lowering=False, debug=False,
                   num_devices=N_CORES)

    xts = [nc.dram_tensor(f"xt{b}", [E, LBS[b]], BF16, kind="ExternalInput").ap()
           for b in range(3)]
    wq = nc.dram_tensor("wq", [E, 128], BF16, kind="ExternalInput").ap()
    wk = nc.dram_tensor("wk", [E, 128], BF16, kind="ExternalInput").ap()
    wv = nc.dram_tensor("wv", [E, 128], BF16, kind="ExternalInput").ap()
    wproj = nc.dram_tensor("wproj", [E, E], BF16, kind="ExternalInput").ap()
    ident = nc.dram_tensor("ident", [128, 128], BF16, kind="ExternalInput").ap()
    trimask = nc.dram_tensor("trimask", [128, 128], BF16, kind="ExternalInput").ap()
    vinv = nc.dram_tensor("vinv", [128, 56], F32, kind="ExternalInput").ap()
    out = nc.dram_tensor("out", [512, E], F32, kind="ExternalOutput").ap()

    from contextlib import ExitStack
    with tile.TileContext(nc) as tc, ExitStack() as stk:
        # ---- persistent pools / folded const loads ---------------------------
        consts = stk.enter_context(tc.tile_pool(name="consts", bufs=1))
        ident_sb = consts.tile([128, 128], BF16)
        tri_sb = consts.tile([128, 128], BF16)
        vinv_sb = consts.tile([128, 56], F32)
        w_sb = {}
        for name, ap in (("q", wq), ("k", wk), ("v", wv)):
            t = consts.tile([128, 8 * 128], BF16, name=f"w{name}sb")
            w_sb[name] = t
        wp_all = consts.tile([128, 8 * E], BF16)

        qkt = stk.enter_context(tc.tile_pool(name="qkt", bufs=1))
        QT = [qkt.tile([128, LBS[b]], BF16, name=f"QT{b}") for b in range(3)]
        KTb = [qkt.tile([128, LBS[b]], BF16, name=f"KT{b}") for b in range(3)]
        vaugp = stk.enter_context(tc.tile_pool(name="vaugp", bufs=1))
        Vaug = [vaugp.tile([128, (LBS[b] // 128) * 130], BF16, name=f"Va{b}")
                for b in range(3)]
        for b in range(3):
            ones3 = Vaug[b][:].rearrange("p (t c) -> p t c", c=65)
            nc.vector.memset(ones3[:, :, 64:65], 1.0)

        dram = stk.enter_context(tc.tile_pool(name="dram", bufs=1, space="DRAM"))
        a2a_in = dram.tile([1024, 512], BF16)
        a2a_out = dram.tile([1024, 512], BF16)
        a2a12_in = dram.tile([1024, 384], BF16)
        a2a12_out = dram.tile([1024, 384], BF16)

        xtp = stk.enter_context(tc.tile_pool(name="xtp", bufs=1))
        esp = stk.enter_context(tc.tile_pool(name="esp", bufs=1))
        otsb = stk.enter_context(tc.tile_pool(name="otsb", bufs=1))
        smallp = stk.enter_context(tc.tile_pool(name="smallp", bufs=1))
        qkvps = stk.enter_context(tc.tile_pool(name="qkvps", bufs=1, space="PSUM"))
        spps = stk.enter_context(tc.tile_pool(name="spps", bufs=1, space="PSUM"))
        otps = stk.enter_context(tc.tile_pool(name="otps", bufs=1, space="PSUM"))

        def load_xt(b, n):
            """Prefetch the window's x^T chunk tiles (depth-2 ahead so no xt
            DMA is ever issued after a collective instruction — DMAs issued
            after a collective wait for its completion)."""
            xts_t = []
            for k in range(8):
                t = xtp.tile([128, G], BF16, tag=f"xt{k}", bufs=3,
                             name=f"xt_t{k}")
                nc.sync.dma_start(t[:], xts[b][128 * k:128 * (k + 1),
                                               G * n:G * (n + 1)])
                xts_t.append(t)
            return xts_t

        def p1_pieces(b, n, xts_t):
            """Closures for the PE-side pieces (2 q-groups, 2 k-groups, 2
            V^T-groups) interleaved between score-phase issues of the
            previous window. V is produced directly in key-major layout by
            flipping the matmul operands (lhsT=x chunk, rhs=w_v), so no
            PE transposes are needed."""

            def qk_piece(s_, nm):
                def emit():
                    ps = qkvps.tile([128, 512], F32, tag="ps", bufs=2,
                                    name="ps")
                    for k in range(8):
                        nc.tensor.matmul(ps[:],
                                         w_sb[nm][:, 128 * k:128 * (k + 1)],
                                         xts_t[k][:, 512 * s_:512 * (s_ + 1)],
                                         start=(k == 0), stop=(k == 7))
                    t_ = QT[b] if nm == "q" else KTb[b]
                    nc.vector.tensor_copy(
                        t_[:, G * n + 512 * s_:G * n + 512 * (s_ + 1)], ps[:])
                return emit

            def vt_piece(s_):
                def emit():
                    ps = qkvps.tile([128, 512], F32, tag="ps", bufs=2,
                                    name="psv")
                    for tl in range(4):
                        t = 4 * s_ + tl
                        for k in range(8):
                            nc.tensor.matmul(
                                ps[:, 128 * tl:128 * (tl + 1)],
                                xts_t[k][:, 128 * t:128 * (t + 1)],
                                w_sb["v"][:, 128 * k:128 * (k + 1)],
                                start=(k == 0), stop=(k == 7))
                    for tl in range(4):
                        t = 4 * s_ + tl
                        vdst = Vaug[b][:, 130 * (8 * n + t):
                                       130 * (8 * n + t + 1)]
                        nc.vector.tensor_copy(
                            vdst.rearrange("p (h c) -> p h c", c=65)
                            [:, :, 0:64],
                            ps[:, 128 * tl:128 * (tl + 1)]
                            .rearrange("p (h c) -> p h c", c=64))
                return emit

            pieces = []
            for s_ in range(2):
                pieces.append(qk_piece(s_, "q"))
                pieces.append(qk_piece(s_, "k"))
                pieces.append(vt_piece(s_))
            return pieces

        def phase1(b, n, pieces=()):
            """Scores + exp for window (b, n), interleaving the next
            window's P1 pieces between key-tile issues to keep the PE fed
            while the Activation engine works through the exponentials."""
            pieces = list(pieces)
            ES = [[None] * 2 for _ in range(KT)]
            for kt in range(KT):
                nq = G - 128 * kt
                base = G * n + 128 * kt
                for hh in range(2):
                    hs = 64 * hh
                    sp = spps.tile([128, 1024], F32, tag="sp", bufs=2,
                                   name="sp")
                    lhsT = KTb[b][hs:hs + 64, base:base + 128]
                    for c0 in range(0, nq, 512):
                        c1 = min(c0 + 512, nq)
                        nc.tensor.matmul(sp[:, c0:c1], lhsT,
                                         QT[b][hs:hs + 64, base + c0:base + c1],
                                         start=True, stop=True)
                    es = esp.tile([128, nq], BF16, tag=f"es{kt}{hh}", bufs=2,
                                  name=f"es{kt}{hh}")
                    nc.scalar.activation(es[:], sp[:, 0:nq],
                                         mybir.ActivationFunctionType.Exp)
                    nc.vector.tensor_mul(es[:, 0:128], es[:, 0:128], tri_sb[:])
                    ES[kt][hh] = es
                if pieces:
                    pieces.pop(0)()
                    if kt >= 4 and pieces:
                        pieces.pop(0)()
            while pieces:
                pieces.pop(0)()
            return ES

        def phase2(b, n, ES):
            ots_w = otsb.tile([128, 1024], BF16, tag="ots", bufs=4,
                              name="ots_w")
            for hh in range(2):
                OT = [otps.tile([128, 512], F32, tag=f"ot{hf}", bufs=1,
                                name=f"OT{hf}") for hf in range(2)]
                for qt in range(KT):
                    o = OT[qt // 4]
                    for kt in range(qt + 1):
                        va = Vaug[b][:, 130 * (KT * n + kt) + 65 * hh:
                                     130 * (KT * n + kt) + 65 * hh + 65]
                        nc.tensor.matmul(
                            o[:, 65 * (qt % 4):65 * (qt % 4) + 65],
                            ES[kt][hh][:, 128 * (qt - kt):128 * (qt - kt) + 128],
                            va, start=(kt == 0), stop=(kt == qt),
                            skip_group_check=True)
                for hf in range(2):
                    o = OT[hf]
                    den = smallp.tile([128, 4], F32, tag=f"dn{hh}{hf}",
                                      bufs=2, name="den")
                    o_g = o[:, 0:260].rearrange("p (t c) -> p t c", c=65)
                    nc.vector.tensor_mul(
                        den[:].rearrange("p (t c) -> p t c", c=1),
                        o_g[:, :, 64:65],
                        vinv_sb[:, TOFF[b] + 8 * n + 4 * hf:
                                TOFF[b] + 8 * n + 4 * hf + 4]
                        .rearrange("p (t c) -> p t c", c=1))
                    sc = smallp.tile([128, 4], F32, tag=f"sc{hh}{hf}",
                                     bufs=2, name="scl")
                    nc.vector.reciprocal_approx_fast(sc[:], den[:])
                    for qp in range(4):
                        qt = 4 * hf + qp
                        nc.vector.tensor_scalar_mul(
                            ots_w[:, 128 * qt + 64 * hh:
                                  128 * qt + 64 * hh + 64],
                            o[:, 65 * qp:65 * qp + 64],
                            sc[:, qp:qp + 1])
            # q-major staging: ship [128 q, 128 feat] pieces as-is; the
            # receiver transposes after the exchange (PE). No DmaTranspose
            # anywhere -> no scheduler serialization against collectives.
            if b == 0:
                for t in range(2):
                    j = 2 * n + t
                    nc.sync.dma_start(a2a_in[128 * j:128 * (j + 1), :],
                                      ots_w[:, 512 * t:512 * (t + 1)])
            elif b == 1:
                for qt in range(KT):
                    j = 4 * n + qt // 2
                    c0 = 128 * (qt % 2)
                    nc.sync.dma_start(
                        a2a12_in[128 * j:128 * (j + 1), c0:c0 + 128],
                        ots_w[:, 128 * qt:128 * (qt + 1)])
            else:
                for qt in range(KT):
                    nc.sync.dma_start(
                        a2a12_in[128 * qt:128 * (qt + 1), 256:384],
                        ots_w[:, 128 * qt:128 * (qt + 1)])

        # ---- software-pipelined steps ----------------------------------------
        sprep = stk.enter_context(tc.tile_pool(name="sprep", bufs=1))
        S = sprep.tile([128, 4096], BF16, name="S")
        nc.vector.memset(S[:], 0.0)
        sprep_tiles = []

        xt_pre = [load_xt(*STEPS[0])]
        boot = p1_pieces(*STEPS[0], xt_pre[0])
        for half in range(2):
            for name, ap in (("q", wq), ("k", wk), ("v", wv)):
                nc.sync.dma_start(
                    w_sb[name][:, 512 * half:512 * (half + 1)]
                    .rearrange("p (k j) -> p k j", k=4),
                    ap[512 * half:512 * (half + 1), :]
                    .rearrange("(k p) j -> p k j", p=128))
        nc.sync.dma_start(ident_sb[:], ident[:])
        nc.sync.dma_start(tri_sb[:], trimask[:])
        nc.sync.dma_start(vinv_sb[:], vinv[:])
        xt_pre.append(load_xt(*STEPS[1]))
        for p in boot:
            p()
        prev = None
        for i, (b, n) in enumerate(STEPS):
            if i + 2 < len(STEPS):
                xt_pre.append(load_xt(*STEPS[i + 2]))
            nxt = (p1_pieces(*STEPS[i + 1], xt_pre[i + 1])
                   if i + 1 < len(STEPS) else [])
            es_now = phase1(b, n, nxt)
            if i == 1:
                nc.sync.dma_start(wp_all[:].rearrange("p (k j) -> p k j", k=8),
                                  wproj.rearrange("(k p) j -> p k j", p=128))
            if prev is not None:
                pb, pn, pes = prev
                phase2(pb, pn, pes)
                if (pb, pn) == (0, 0):
                    # textually late: every DMA issued after a collective
                    # waits for it; here only naturally-late stores follow.
                    # Its start is dep-driven (b1/b2 staging), so it still
                    # overlaps the branch-0 steps.
                    nc.gpsimd.collective_compute(
                        "AllToAll", mybir.AluOpType.bypass,
                        replica_groups=[list(range(N_CORES))],
                        ins=[a2a12_in.opt()], outs=[a2a12_out.opt()])
                if (pb, pn) == (0, 1):
                    # a2a12 results: loads issued here (blocked only until
                    # the collective completes, harmless on the DMA queue)
                    T1 = sprep.tile([128, 8 * 256], BF16, name="T1")
                    nc.sync.dma_start(T1[:].rearrange("p (k j) -> p k j", k=8),
                                      a2a12_out[:, 0:256]
                                      .rearrange("(k p) j -> p k j", p=128))
                    T2 = sprep.tile([128, 8 * 128], BF16, name="T2")
                    nc.sync.dma_start(T2[:].rearrange("p (k j) -> p k j", k=8),
                                      a2a12_out[:, 256:384]
                                      .rearrange("(k p) j -> p k j", p=128))
                    sprep_tiles.extend([T1, T2])
                if (pb, pn) == (0, 2):
                    # prebuild the sparse branch sum S one step later so the
                    # PE never queues behind the collective-gated loads:
                    # transpose each received [128 q, 128 f] piece on the PE
                    # and scatter straight from PSUM into S.
                    T1, T2 = sprep_tiles

                    def tr_slot_pool():
                        state = {}
                        def get():
                            i = state.get("i", 0)
                            if i % 4 == 0:
                                state["t"] = qkvps.tile(
                                    [128, 512], F32, tag="ps", bufs=2,
                                    name="pst")
                            state["i"] = i + 1
                            sl = state["t"][:, 64 * (i % 4):64 * (i % 4) + 64]
                            return sl.bitcast(BF16)
                        return get
                    slot = tr_slot_pool()
                    for jj in range(8):
                        i2, i4 = jj // 4, jj // 2
                        for s_ in range(2):
                            ptr = slot()
                            nc.tensor.transpose(
                                ptr,
                                T1[:, 256 * jj + 128 * s_:
                                   256 * jj + 128 * (s_ + 1)],
                                ident_sb[:])
                            sl = S[:, 512 * jj + 256 * s_:
                                   512 * jj + 256 * (s_ + 1)]
                            nc.vector.tensor_copy(
                                sl.rearrange("p (t c) -> p t c", c=2)
                                [:, :, i2:i2 + 1],
                                ptr.rearrange("p (t c) -> p t c", c=1))
                        ptr = slot()
                        nc.tensor.transpose(ptr,
                                            T2[:, 128 * jj:128 * (jj + 1)],
                                            ident_sb[:])
                        s4 = S[:, 512 * jj:512 * (jj + 1)] \
                            .rearrange("p (t c) -> p t c", c=4)
                        nc.vector.tensor_add(
                            s4[:, :, i4:i4 + 1], s4[:, :, i4:i4 + 1],
                            ptr.rearrange("p (t c) -> p t c", c=1))
            prev = (b, n, es_now)
        phase2(*prev)
        nc.gpsimd.collective_compute(
            "AllToAll", mybir.AluOpType.bypass,
            replica_groups=[list(range(N_CORES))],
            ins=[a2a_in.opt()], outs=[a2a_out.opt()])

        # ---- P4: dense slice + prebuilt sparse sum ----------------------------
        with (tc.tile_pool(name="ptp", bufs=1) as ptp,
              tc.tile_pool(name="ocp", bufs=1) as ocp):
            PTq = ptp.tile([128, 8 * 512], BF16, name="PTq")
            for jj in range(8):
                nc.sync.dma_start(PTq[:, 512 * jj:512 * (jj + 1)],
                                  a2a_out[128 * jj:128 * (jj + 1), :])
            PT = ptp.tile([128, 8 * 512], BF16, name="PT")
            # s_-major so P5's m=s_ accumulations can start after one batch;
            # the branch-sum S is fused into the PSUM->PT move.
            for s_ in range(4):
                for g in range(2):
                    pst2 = qkvps.tile([128, 512], F32, tag="ps", bufs=2,
                                      name="pst2")
                    for q_ in range(4):
                        jj = 4 * g + q_
                        ptr = pst2[:, 64 * q_:64 * (q_ + 1)].bitcast(BF16)
                        nc.tensor.transpose(
                            ptr, PTq[:, 512 * jj + 128 * s_:
                                     512 * jj + 128 * (s_ + 1)], ident_sb[:])
                    # one grouped add for the 4 transposed pieces
                    pt4 = PT[:].rearrange("p (t s c) -> p t s c", s=4, c=128)
                    s4_ = S[:].rearrange("p (t s c) -> p t s c", s=4, c=128)
                    nc.vector.tensor_add(
                        pt4[:, 4 * g:4 * g + 4, s_:s_ + 1, :],
                        pst2[:, 0:256].bitcast(BF16)
                        .rearrange("p (t s c) -> p t s c", s=1, c=128),
                        s4_[:, 4 * g:4 * g + 4, s_:s_ + 1, :])

            # ---- P5: projection (folded output store) -------------------------
            oc_all = ocp.tile([128, 4096], F32, name="oc_all")
            for m in range(4):
                for nb in range(2):
                    pp = qkvps.tile([128, 512], F32, tag="ps", bufs=2,
                                    name="pp")
                    for jj in range(8):
                        nc.tensor.matmul(
                            pp[:], PT[:, 512 * jj + 128 * m:
                                       512 * jj + 128 * (m + 1)],
                            wp_all[:, E * jj + 512 * nb:E * jj + 512 * (nb + 1)],
                            start=(jj == 0), stop=(jj == 7))
                    nc.vector.tensor_copy(
                        oc_all[:, 1024 * m + 512 * nb:1024 * m + 512 * (nb + 1)],
                        pp[:])
                nc.sync.dma_start(out[128 * m:128 * (m + 1), :],
                                  oc_all[:, 1024 * m:1024 * (m + 1)])
    nc.compile()
    return nc


_NC_CACHE = None


def _get_nc():
    global _NC_CACHE
    if _NC_CACHE is None:
        _NC_CACHE = build_nc()
    return _NC_CACHE


def _host_inputs(x, w_qkv, w_proj):
    xT = np.ascontiguousarray(x[0].T).astype(np.float32)      # (E, L)
    ident = np.eye(128, dtype=np.float32)
    import ml_dtypes
    f = np.arange(128)
    trimask = np.where(f[None, :] >= f[:, None], 1.0, 0.0).astype(ml_dtypes.bfloat16)
    in_maps = []
    for c in range(N_CORES):
        h = 2 * c
        vps = []
        for b, r in enumerate(RATIOS):
            i = h // (16 // r)
            cs = r * np.arange(L // r) + i
            V = 1 + (cs % 2 == h // 8).astype(np.int32) \
                  + (cs % 4 == h // 4).astype(np.int32)
            vps.append(V.astype(np.float32))
        vinv2 = np.concatenate(vps).reshape(56, 128).T        # (128, 56)
        i2, i4 = c // 4, c // 2
        m = {
            "xt0": xT,
            "xt1": np.ascontiguousarray(xT[:, i2::2]),
            "xt2": np.ascontiguousarray(xT[:, i4::4]),
            "wq": np.ascontiguousarray(w_qkv[:, 128 * c:128 * (c + 1)]) / 8.0,
            "wk": np.ascontiguousarray(w_qkv[:, E + 128 * c:E + 128 * (c + 1)]),
            "wv": np.ascontiguousarray(w_qkv[:, 2 * E + 128 * c:2 * E + 128 * (c + 1)]),
            "wproj": np.ascontiguousarray(w_proj),
            "ident": ident,
            "trimask": trimask,
            "vinv": np.ascontiguousarray(vinv2),
        }
        bf = ("trimask", "ident", "xt0", "xt1", "xt2", "wq", "wk", "wv", "wproj")
        in_maps.append({k: np.ascontiguousarray(
                            v if k == "trimask" else
                            np.asarray(v, np.float32).astype(ml_dtypes.bfloat16))
                        if k in bf
                        else np.ascontiguousarray(v, dtype=np.float32)
                        for k, v in m.items()})
    return in_maps


def kernel(x, w_qkv, w_proj, _trace=False):
    x = np.asarray(x, np.float32)
    w_qkv = np.asarray(w_qkv, np.float32)
    w_proj = np.asarray(w_proj, np.float32)
    nc = _get_nc()
    in_maps = _host_inputs(x, w_qkv, w_proj)
    res = run_bass_kernel_spmd(nc, in_maps, core_ids=list(range(N_CORES)),
                               trace=_trace)
    full = np.empty((L, E), np.float32)
    for c in range(N_CORES):
        full[512 * c:512 * (c + 1)] = res.results[c]["out"]
    out = full.reshape(1, L, E)
    if _trace:
        return out, res
    return out


# revision 37
# speedup vs baseline: 237.4370x; 1.0063x over previous
"""Trainium2 Bass kernel for DilatedCausalSelfAttention (B=1, L=4096, E=1024,
16 heads, d=64; branches (w,r) = (1024,1), (2048,2), (4096,4)).

Distribution: head-sharded, 2 heads per core (core c owns heads 2c, 2c+1).
Host pre-gathers x^T columns per branch (phase offsets i2=c//4, i4=c//2 are
per-core data, so gathering must happen host-side for an SPMD program).

Software-pipelined window loop (windows processed b=2, b=1 x2, b=0 x4 so the
branch-1/2 AllToAll overlaps branch-0 compute). Per step: P1 (QKV matmuls for
this window, V transposed into key-major V_aug with a ones column), score
phase (K^T Q -> exp -> es bf16, kept in SBUF), then PV + tail of the
PREVIOUS window — so the PE works on P1/QK matmuls while the Activation
engine chews the previous window's exponentials.

PV uses lhsT=es blocks against rhs=V_aug giving O^T [128 q, 65] PSUM groups
(softmax denominator lands in col 64 as a per-partition scalar; one
accumulation group open per PSUM bank at a time — hardware discards an open
group's pending data if another group starts in the same bank). Tail:
scl = 1/(denom*V) via tensor_scalar, both heads into one [128 q, 128 feat]
bf16 tile per q-tile, DMA-transposed to [feat, q] and staged straight into
the AllToAll DRAM buffers. Two AllToAlls redistribute attn^T; each core then
merges branch pieces (strided adds keyed by source block index) and projects
its 512 rows. All DMAs are batched (HWDGE costs ~625ns per instruction).

The branch combination weights equal 1/V(p,h) (V = #branches covering the
position) to ~2e-4 because the reference's probs-logsumexp weights are
log(g + delta) with g=1024, delta in [1, 1.72].
"""

import numpy as np

import concourse.bacc as bacc
import concourse.tile as tile
from concourse import mybir
from concourse.bass_utils import run_bass_kernel_spmd

F32 = mybir.dt.float32
BF16 = mybir.dt.bfloat16

N_CORES = 8
L = 4096
E = 1024
D = 64
G = 1024
KT = 8
RATIOS = [1, 2, 4]
LBS = [L // r for r in RATIOS]
NWIN = [lb // G for lb in LBS]
TOFF = [0, 32, 48]
STEPS = [(2, 0), (1, 0), (1, 1), (0, 0), (0, 1), (0, 2), (0, 3)]


def build_nc():
    nc = bacc.Bacc("TRN2", target_bir_lowering=False, debug=False,
                   num_devices=N_CORES)

    xts = [nc.dram_tensor(f"xt{b}", [E, LBS[b]], BF16, kind="ExternalInput").ap()
           for b in range(3)]
    wq = nc.dram_tensor("wq", [E, 128], BF16, kind="ExternalInput").ap()
    wk = nc.dram_tensor("wk", [E, 128], BF16, kind="ExternalInput").ap()
    wv = nc.dram_tensor("wv", [E, 128], BF16, kind="ExternalInput").ap()
    wproj = nc.dram_tensor("wproj", [E, E], BF16, kind="ExternalInput").ap()
    ident = nc.dram_tensor("ident", [128, 128], BF16, kind="ExternalInput").ap()
    trimask = nc.dram_tensor("trimask", [128, 128], BF16, kind="ExternalInput").ap()
    vinv = nc.dram_tensor("vinv", [128, 56], F32, kind="ExternalInput").ap()
    out = nc.dram_tensor("out", [512, E], F32, kind="ExternalOutput").ap()

    from contextlib import ExitStack
    with tile.TileContext(nc) as tc, ExitStack() as stk:
        # ---- persistent pools / folded const loads ---------------------------
        consts = stk.enter_context(tc.tile_pool(name="consts", bufs=1))
        ident_sb = consts.tile([128, 128], BF16)
        tri_sb = consts.tile([128, 128], BF16)
        vinv_sb = consts.tile([128, 56], F32)
        w_sb = {}
        for name, ap in (("q", wq), ("k", wk), ("v", wv)):
            t = consts.tile([128, 8 * 128], BF16, name=f"w{name}sb")
            w_sb[name] = t
        wp_all = consts.tile([128, 8 * E], BF16)

        qkt = stk.enter_context(tc.tile_pool(name="qkt", bufs=1))
        QT = [qkt.tile([128, LBS[b]], BF16, name=f"QT{b}") for b in range(3)]
        KTb = [qkt.tile([128, LBS[b]], BF16, name=f"KT{b}") for b in range(3)]
        vaugp = stk.enter_context(tc.tile_pool(name="vaugp", bufs=1))
        Vaug = [vaugp.tile([128, (LBS[b] // 128) * 130], BF16, name=f"Va{b}")
                for b in range(3)]
        for b in range(3):
            ones3 = Vaug[b][:].rearrange("p (t c) -> p t c", c=65)
            nc.vector.memset(ones3[:, :, 64:65], 1.0)

        dram = stk.enter_context(tc.tile_pool(name="dram", bufs=1, space="DRAM"))
        a2a_in = dram.tile([1024, 512], BF16)
        a2a_out = dram.tile([1024, 512], BF16)
        a2a12_in = dram.tile([1024, 384], BF16)
        a2a12_out = dram.tile([1024, 384], BF16)

        xtp = stk.enter_context(tc.tile_pool(name="xtp", bufs=1))
        esp = stk.enter_context(tc.tile_pool(name="esp", bufs=1))
        otsb = stk.enter_context(tc.tile_pool(name="otsb", bufs=1))
        smallp = stk.enter_context(tc.tile_pool(name="smallp", bufs=1))
        qkvps = stk.enter_context(tc.tile_pool(name="qkvps", bufs=1, space="PSUM"))
        spps = stk.enter_context(tc.tile_pool(name="spps", bufs=1, space="PSUM"))
        otps = stk.enter_context(tc.tile_pool(name="otps", bufs=1, space="PSUM"))

        def load_xt(b, n):
            """Prefetch the window's x^T chunk tiles (depth-2 ahead so no xt
            DMA is ever issued after a collective instruction — DMAs issued
            after a collective wait for its completion)."""
            xts_t = []
            for k in range(8):
                t = xtp.tile([128, G], BF16, tag=f"xt{k}", bufs=3,
                             name=f"xt_t{k}")
                nc.sync.dma_start(t[:], xts[b][128 * k:128 * (k + 1),
                                               G * n:G * (n + 1)])
                xts_t.append(t)
            return xts_t

        def p1_pieces(b, n, xts_t):
            """Closures for the PE-side pieces (2 q-groups, 2 k-groups, 2
            V^T-groups) interleaved between score-phase issues of the
            previous window. V is produced directly in key-major layout by
            flipping the matmul operands (lhsT=x chunk, rhs=w_v), so no
            PE transposes are needed."""

            def qk_piece(s_, nm):
                def emit():
                    ps = qkvps.tile([128, 512], F32, tag="ps", bufs=2,
                                    name="ps")
                    for k in range(8):
                        nc.tensor.matmul(ps[:],
                                         w_sb[nm][:, 128 * k:128 * (k + 1)],
                                         xts_t[k][:, 512 * s_:512 * (s_ + 1)],
                                         start=(k == 0), stop=(k == 7))
                    t_ = QT[b] if nm == "q" else KTb[b]
                    nc.vector.tensor_copy(
                        t_[:, G * n + 512 * s_:G * n + 512 * (s_ + 1)], ps[:])
                return emit

            def vt_piece(s_):
                def emit():
                    ps = qkvps.tile([128, 512], F32, tag="ps", bufs=2,
                                    name="psv")
                    for tl in range(4):
                        t = 4 * s_ + tl
                        for k in range(8):
                            nc.tensor.matmul(
                                ps[:, 128 * tl:128 * (tl + 1)],
                                xts_t[k][:, 128 * t:128 * (t + 1)],
                                w_sb["v"][:, 128 * k:128 * (k + 1)],
                                start=(k == 0), stop=(k == 7))
                    for tl in range(4):
                        t = 4 * s_ + tl
                        vdst = Vaug[b][:, 130 * (8 * n + t):
                                       130 * (8 * n + t + 1)]
                        nc.vector.tensor_copy(
                            vdst.rearrange("p (h c) -> p h c", c=65)
                            [:, :, 0:64],
                            ps[:, 128 * tl:128 * (tl + 1)]
                            .rearrange("p (h c) -> p h c", c=64))
                return emit

            pieces = []
            for s_ in range(2):
                pieces.append(qk_piece(s_, "q"))
                pieces.append(qk_piece(s_, "k"))
                pieces.append(vt_piece(s_))
            return pieces

        KT_GROUPS = [(0,), (1,), (2,), (3,), (4, 5), (6, 7)]

        def phase1(b, n, pieces=()):
            """Scores + exp for window (b, n), interleaving the next
            window's P1 pieces between key-tile issues to keep the PE fed
            while the Activation engine works through the exponentials.
            Small key-tiles (kt 4+5, 6+7) share one score tile and one
            exp instruction to amortize the Activation per-op overhead."""
            pieces = list(pieces)
            ES = {}
            for gi, kts in enumerate(KT_GROUPS):
                for hh in range(2):
                    hs = 64 * hh
                    sp = spps.tile([128, 1024], F32, tag="sp", bufs=2,
                                   name="sp")
                    off = 0
                    offs = []
                    for kt in kts:
                        nq = G - 128 * kt
                        base = G * n + 128 * kt
                        lhsT = KTb[b][hs:hs + 64, base:base + 128]
                        for c0 in range(0, nq, 512):
                            c1 = min(c0 + 512, nq)
                            nc.tensor.matmul(
                                sp[:, off + c0:off + c1], lhsT,
                                QT[b][hs:hs + 64, base + c0:base + c1],
                                start=True, stop=True)
                        offs.append(off)
                        off += nq
                    es = esp.tile([128, off], BF16, tag=f"esg{gi}{hh}",
                                  bufs=2, name=f"esg{gi}{hh}")
                    nc.scalar.activation(es[:], sp[:, 0:off],
                                         mybir.ActivationFunctionType.Exp)
                    for kt, o in zip(kts, offs):
                        nc.vector.tensor_mul(es[:, o:o + 128],
                                             es[:, o:o + 128], tri_sb[:])
                        ES[(kt, hh)] = (es, o)
                if pieces:
                    pieces.pop(0)()
                    if gi >= 3 and pieces:
                        pieces.pop(0)()
            while pieces:
                pieces.pop(0)()
            return ES

        def phase2(b, n, ES):
            ots_w = otsb.tile([128, 1024], BF16, tag="ots", bufs=4,
                              name="ots_w")
            for hh in range(2):
                OT = [otps.tile([128, 512], F32, tag=f"ot{hf}", bufs=1,
                                name=f"OT{hf}") for hf in range(2)]
                for qt in range(KT):
                    o = OT[qt // 4]
                    for kt in range(qt + 1):
                        va = Vaug[b][:, 130 * (KT * n + kt) + 65 * hh:
                                     130 * (KT * n + kt) + 65 * hh + 65]
                        est, eo = ES[(kt, hh)]
                        nc.tensor.matmul(
                            o[:, 65 * (qt % 4):65 * (qt % 4) + 65],
                            est[:, eo + 128 * (qt - kt):
                                eo + 128 * (qt - kt) + 128],
                            va, start=(kt == 0), stop=(kt == qt),
                            skip_group_check=True)
                for hf in range(2):
                    o = OT[hf]
                    den = smallp.tile([128, 4], F32, tag=f"dn{hh}{hf}",
                                      bufs=2, name="den")
                    o_g = o[:, 0:260].rearrange("p (t c) -> p t c", c=65)
                    nc.vector.tensor_mul(
                        den[:].rearrange("p (t c) -> p t c", c=1),
                        o_g[:, :, 64:65],
                        vinv_sb[:, TOFF[b] + 8 * n + 4 * hf:
                                TOFF[b] + 8 * n + 4 * hf + 4]
                        .rearrange("p (t c) -> p t c", c=1))
                    sc = smallp.tile([128, 4], F32, tag=f"sc{hh}{hf}",
                                     bufs=2, name="scl")
                    nc.vector.reciprocal_approx_fast(sc[:], den[:])
                    for qp in range(4):
                        qt = 4 * hf + qp
                        nc.vector.tensor_scalar_mul(
                            ots_w[:, 128 * qt + 64 * hh:
                                  128 * qt + 64 * hh + 64],
                            o[:, 65 * qp:65 * qp + 64],
                            sc[:, qp:qp + 1])
            # q-major staging: ship [128 q, 128 feat] pieces as-is; the
            # receiver transposes after the exchange (PE). No DmaTranspose
            # anywhere -> no scheduler serialization against collectives.
            if b == 0:
                for t in range(2):
                    j = 2 * n + t
                    nc.sync.dma_start(a2a_in[128 * j:128 * (j + 1), :],
                                      ots_w[:, 512 * t:512 * (t + 1)])
            elif b == 1:
                for qt in range(KT):
                    j = 4 * n + qt // 2
                    c0 = 128 * (qt % 2)
                    nc.sync.dma_start(
                        a2a12_in[128 * j:128 * (j + 1), c0:c0 + 128],
                        ots_w[:, 128 * qt:128 * (qt + 1)])
            else:
                for qt in range(KT):
                    nc.sync.dma_start(
                        a2a12_in[128 * qt:128 * (qt + 1), 256:384],
                        ots_w[:, 128 * qt:128 * (qt + 1)])

        # ---- software-pipelined steps ----------------------------------------
        sprep = stk.enter_context(tc.tile_pool(name="sprep", bufs=1))
        S = sprep.tile([128, 4096], BF16, name="S")
        nc.vector.memset(S[:], 0.0)
        sprep_tiles = []

        xt_pre = [load_xt(*STEPS[0])]
        boot = p1_pieces(*STEPS[0], xt_pre[0])
        for half in range(2):
            for name, ap in (("q", wq), ("k", wk), ("v", wv)):
                nc.sync.dma_start(
                    w_sb[name][:, 512 * half:512 * (half + 1)]
                    .rearrange("p (k j) -> p k j", k=4),
                    ap[512 * half:512 * (half + 1), :]
                    .rearrange("(k p) j -> p k j", p=128))
        nc.sync.dma_start(ident_sb[:], ident[:])
        nc.sync.dma_start(tri_sb[:], trimask[:])
        nc.sync.dma_start(vinv_sb[:], vinv[:])
        xt_pre.append(load_xt(*STEPS[1]))
        for p in boot:
            p()
        prev = None
        for i, (b, n) in enumerate(STEPS):
            if i + 2 < len(STEPS):
                xt_pre.append(load_xt(*STEPS[i + 2]))
            nxt = (p1_pieces(*STEPS[i + 1], xt_pre[i + 1])
                   if i + 1 < len(STEPS) else [])
            es_now = phase1(b, n, nxt)
            if i == 1:
                nc.sync.dma_start(wp_all[:].rearrange("p (k j) -> p k j", k=8),
                                  wproj.rearrange("(k p) j -> p k j", p=128))
            if prev is not None:
                pb, pn, pes = prev
                phase2(pb, pn, pes)
                if (pb, pn) == (0, 0):
                    # textually late: every DMA issued after a collective
                    # waits for it; here only naturally-late stores follow.
                    # Its start is dep-driven (b1/b2 staging), so it still
                    # overlaps the branch-0 steps.
                    nc.gpsimd.collective_compute(
                        "AllToAll", mybir.AluOpType.bypass,
                        replica_groups=[list(range(N_CORES))],
                        ins=[a2a12_in.opt()], outs=[a2a12_out.opt()])
                if (pb, pn) == (0, 1):
                    # a2a12 results: loads issued here (blocked only until
                    # the collective completes, harmless on the DMA queue)
                    T1 = sprep.tile([128, 8 * 256], BF16, name="T1")
                    nc.sync.dma_start(T1[:].rearrange("p (k j) -> p k j", k=8),
                                      a2a12_out[:, 0:256]
                                      .rearrange("(k p) j -> p k j", p=128))
                    T2 = sprep.tile([128, 8 * 128], BF16, name="T2")
                    nc.sync.dma_start(T2[:].rearrange("p (k j) -> p k j", k=8),
                                      a2a12_out[:, 256:384]
                                      .rearrange("(k p) j -> p k j", p=128))
                    sprep_tiles.extend([T1, T2])
                if (pb, pn) == (0, 2):
                    # prebuild the sparse branch sum S one step later so the
                    # PE never queues behind the collective-gated loads:
                    # transpose each received [128 q, 128 f] piece on the PE
                    # and scatter straight from PSUM into S.
                    T1, T2 = sprep_tiles

                    def tr_slot_pool():
                        state = {}
                        def get():
                            i = state.get("i", 0)
                            if i % 4 == 0:
                                state["t"] = qkvps.tile(
                                    [128, 512], F32, tag="ps", bufs=2,
                                    name="pst")
                            state["i"] = i + 1
                            sl = state["t"][:, 64 * (i % 4):64 * (i % 4) + 64]
                            return sl.bitcast(BF16)
                        return get
                    slot = tr_slot_pool()
                    for jj in range(8):
                        i2, i4 = jj // 4, jj // 2
                        for s_ in range(2):
                            ptr = slot()
                            nc.tensor.transpose(
                                ptr,
                                T1[:, 256 * jj + 128 * s_:
                                   256 * jj + 128 * (s_ + 1)],
                                ident_sb[:])
                            sl = S[:, 512 * jj + 256 * s_:
                                   512 * jj + 256 * (s_ + 1)]
                            nc.vector.tensor_copy(
                                sl.rearrange("p (t c) -> p t c", c=2)
                                [:, :, i2:i2 + 1],
                                ptr.rearrange("p (t c) -> p t c", c=1))
                        ptr = slot()
                        nc.tensor.transpose(ptr,
                                            T2[:, 128 * jj:128 * (jj + 1)],
                                            ident_sb[:])
                        s4 = S[:, 512 * jj:512 * (jj + 1)] \
                            .rearrange("p (t c) -> p t c", c=4)
                        nc.vector.tensor_add(
                            s4[:, :, i4:i4 + 1], s4[:, :, i4:i4 + 1],
                            ptr.rearrange("p (t c) -> p t c", c=1))
            prev = (b, n, es_now)
        phase2(*prev)
        nc.gpsimd.collective_compute(
            "AllToAll", mybir.AluOpType.bypass,
            replica_groups=[list(range(N_CORES))],
            ins=[a2a_in.opt()], outs=[a2a_out.opt()])

        # ---- P4: dense slice + prebuilt sparse sum ----------------------------
        with (tc.tile_pool(name="ptp", bufs=1) as ptp,
              tc.tile_pool(name="ocp", bufs=1) as ocp):
            PTq = ptp.tile([128, 8 * 512], BF16, name="PTq")
            for jj in range(8):
                nc.sync.dma_start(PTq[:, 512 * jj:512 * (jj + 1)],
                                  a2a_out[128 * jj:128 * (jj + 1), :])
            PT = ptp.tile([128, 8 * 512], BF16, name="PT")
            # s_-major so P5's m=s_ accumulations can start after one batch;
            # the branch-sum S is fused into the PSUM->PT move.
            for s_ in range(4):
                for g in range(2):
                    pst2 = qkvps.tile([128, 512], F32, tag="ps", bufs=2,
                                      name="pst2")
                    for q_ in range(4):
                        jj = 4 * g + q_
                        ptr = pst2[:, 64 * q_:64 * (q_ + 1)].bitcast(BF16)
                        nc.tensor.transpose(
                            ptr, PTq[:, 512 * jj + 128 * s_:
                                     512 * jj + 128 * (s_ + 1)], ident_sb[:])
                    # one grouped add for the 4 transposed pieces
                    pt4 = PT[:].rearrange("p (t s c) -> p t s c", s=4, c=128)
                    s4_ = S[:].rearrange("p (t s c) -> p t s c", s=4, c=128)
                    nc.vector.tensor_add(
                        pt4[:, 4 * g:4 * g + 4, s_:s_ + 1, :],
                        pst2[:, 0:256].bitcast(BF16)
                        .rearrange("p (t s c) -> p t s c", s=1, c=128),
                        s4_[:, 4 * g:4 * g + 4, s_:s_ + 1, :])

            # ---- P5: projection (folded output store) -------------------------
            oc_all = ocp.tile([128, 4096], F32, name="oc_all")
            for m in range(4):
                for nb in range(2):
                    pp = qkvps.tile([128, 512], F32, tag="ps", bufs=2,
                                    name="pp")
                    for jj in range(8):
                        nc.tensor.matmul(
                            pp[:], PT[:, 512 * jj + 128 * m:
                                       512 * jj + 128 * (m + 1)],
                            wp_all[:, E * jj + 512 * nb:E * jj + 512 * (nb + 1)],
                            start=(jj == 0), stop=(jj == 7))
                    nc.vector.tensor_copy(
                        oc_all[:, 1024 * m + 512 * nb:1024 * m + 512 * (nb + 1)],
                        pp[:])
                nc.sync.dma_start(out[128 * m:128 * (m + 1), :],
                                  oc_all[:, 1024 * m:1024 * (m + 1)])
    nc.compile()
    return nc


_NC_CACHE = None


def _get_nc():
    global _NC_CACHE
    if _NC_CACHE is None:
        _NC_CACHE = build_nc()
    return _NC_CACHE


def _host_inputs(x, w_qkv, w_proj):
    xT = np.ascontiguousarray(x[0].T).astype(np.float32)      # (E, L)
    ident = np.eye(128, dtype=np.float32)
    import ml_dtypes
    f = np.arange(128)
    trimask = np.where(f[None, :] >= f[:, None], 1.0, 0.0).astype(ml_dtypes.bfloat16)
    in_maps = []
    for c in range(N_CORES):
        h = 2 * c
        vps = []
        for b, r in enumerate(RATIOS):
            i = h // (16 // r)
            cs = r * np.arange(L // r) + i
            V = 1 + (cs % 2 == h // 8).astype(np.int32) \
                  + (cs % 4 == h // 4).astype(np.int32)
            vps.append(V.astype(np.float32))
        vinv2 = np.concatenate(vps).reshape(56, 128).T        # (128, 56)
        i2, i4 = c // 4, c // 2
        m = {
            "xt0": xT,
            "xt1": np.ascontiguousarray(xT[:, i2::2]),
            "xt2": np.ascontiguousarray(xT[:, i4::4]),
            "wq": np.ascontiguousarray(w_qkv[:, 128 * c:128 * (c + 1)]) / 8.0,
            "wk": np.ascontiguousarray(w_qkv[:, E + 128 * c:E + 128 * (c + 1)]),
            "wv": np.ascontiguousarray(w_qkv[:, 2 * E + 128 * c:2 * E + 128 * (c + 1)]),
            "wproj": np.ascontiguousarray(w_proj),
            "ident": ident,
            "trimask": trimask,
            "vinv": np.ascontiguousarray(vinv2),
        }
        bf = ("trimask", "ident", "xt0", "xt1", "xt2", "wq", "wk", "wv", "wproj")
        in_maps.append({k: np.ascontiguousarray(
                            v if k == "trimask" else
                            np.asarray(v, np.float32).astype(ml_dtypes.bfloat16))
                        if k in bf
                        else np.ascontiguousarray(v, dtype=np.float32)
                        for k, v in m.items()})
    return in_maps


def kernel(x, w_qkv, w_proj, _trace=False):
    x = np.asarray(x, np.float32)
    w_qkv = np.asarray(w_qkv, np.float32)
    w_proj = np.asarray(w_proj, np.float32)
    nc = _get_nc()
    in_maps = _host_inputs(x, w_qkv, w_proj)
    res = run_bass_kernel_spmd(nc, in_maps, core_ids=list(range(N_CORES)),
                               trace=_trace)
    full = np.empty((L, E), np.float32)
    for c in range(N_CORES):
        full[512 * c:512 * (c + 1)] = res.results[c]["out"]
    out = full.reshape(1, L, E)
    if _trace:
        return out, res
    return out


# revision 38
# speedup vs baseline: 239.4323x; 1.0084x over previous
"""Trainium2 Bass kernel for DilatedCausalSelfAttention (B=1, L=4096, E=1024,
16 heads, d=64; branches (w,r) = (1024,1), (2048,2), (4096,4)).

Distribution: head-sharded, 2 heads per core (core c owns heads 2c, 2c+1).
Host pre-gathers x^T columns per branch (phase offsets i2=c//4, i4=c//2 are
per-core data, so gathering must happen host-side for an SPMD program).

Software-pipelined window loop (windows processed b=2, b=1 x2, b=0 x4 so the
branch-1/2 AllToAll overlaps branch-0 compute). Per step: P1 (QKV matmuls for
this window, V transposed into key-major V_aug with a ones column), score
phase (K^T Q -> exp -> es bf16, kept in SBUF), then PV + tail of the
PREVIOUS window — so the PE works on P1/QK matmuls while the Activation
engine chews the previous window's exponentials.

PV uses lhsT=es blocks against rhs=V_aug giving O^T [128 q, 65] PSUM groups
(softmax denominator lands in col 64 as a per-partition scalar; one
accumulation group open per PSUM bank at a time — hardware discards an open
group's pending data if another group starts in the same bank). Tail:
scl = 1/(denom*V) via tensor_scalar, both heads into one [128 q, 128 feat]
bf16 tile per q-tile, DMA-transposed to [feat, q] and staged straight into
the AllToAll DRAM buffers. Two AllToAlls redistribute attn^T; each core then
merges branch pieces (strided adds keyed by source block index) and projects
its 512 rows. All DMAs are batched (HWDGE costs ~625ns per instruction).

The branch combination weights equal 1/V(p,h) (V = #branches covering the
position) to ~2e-4 because the reference's probs-logsumexp weights are
log(g + delta) with g=1024, delta in [1, 1.72].
"""

import numpy as np

import concourse.bacc as bacc
import concourse.tile as tile
from concourse import mybir
from concourse.bass_utils import run_bass_kernel_spmd

F32 = mybir.dt.float32
BF16 = mybir.dt.bfloat16

N_CORES = 8
L = 4096
E = 1024
D = 64
G = 1024
KT = 8
RATIOS = [1, 2, 4]
LBS = [L // r for r in RATIOS]
NWIN = [lb // G for lb in LBS]
TOFF = [0, 32, 48]
STEPS = [(2, 0), (1, 0), (1, 1), (0, 0), (0, 1), (0, 2), (0, 3)]


def build_nc():
    nc = bacc.Bacc("TRN2", target_bir_lowering=False, debug=False,
                   num_devices=N_CORES)

    xts = [nc.dram_tensor(f"xt{b}", [E, LBS[b]], BF16, kind="ExternalInput").ap()
           for b in range(3)]
    wq = nc.dram_tensor("wq", [E, 128], BF16, kind="ExternalInput").ap()
    wk = nc.dram_tensor("wk", [E, 128], BF16, kind="ExternalInput").ap()
    wv = nc.dram_tensor("wv", [E, 128], BF16, kind="ExternalInput").ap()
    wproj = nc.dram_tensor("wproj", [E, E], BF16, kind="ExternalInput").ap()
    ident = nc.dram_tensor("ident", [128, 128], BF16, kind="ExternalInput").ap()
    trimask = nc.dram_tensor("trimask", [128, 128], BF16, kind="ExternalInput").ap()
    vinv = nc.dram_tensor("vinv", [128, 56], F32, kind="ExternalInput").ap()
    out = nc.dram_tensor("out", [512, E], F32, kind="ExternalOutput").ap()

    from contextlib import ExitStack
    with tile.TileContext(nc) as tc, ExitStack() as stk:
        # ---- persistent pools / folded const loads ---------------------------
        consts = stk.enter_context(tc.tile_pool(name="consts", bufs=1))
        ident_sb = consts.tile([128, 128], BF16)
        tri_sb = consts.tile([128, 128], BF16)
        vinv_sb = consts.tile([128, 56], F32)
        w_sb = {}
        for name, ap in (("q", wq), ("k", wk), ("v", wv)):
            t = consts.tile([128, 8 * 128], BF16, name=f"w{name}sb")
            w_sb[name] = t
        wp_all = consts.tile([128, 8 * E], BF16)

        qkt = stk.enter_context(tc.tile_pool(name="qkt", bufs=1))
        QT = [qkt.tile([128, LBS[b]], BF16, name=f"QT{b}") for b in range(3)]
        KTb = [qkt.tile([128, LBS[b]], BF16, name=f"KT{b}") for b in range(3)]
        vaugp = stk.enter_context(tc.tile_pool(name="vaugp", bufs=1))
        Vaug = [vaugp.tile([128, (LBS[b] // 128) * 130], BF16, name=f"Va{b}")
                for b in range(3)]
        for b in range(3):
            ones3 = Vaug[b][:].rearrange("p (t c) -> p t c", c=65)
            nc.vector.memset(ones3[:, :, 64:65], 1.0)

        dram = stk.enter_context(tc.tile_pool(name="dram", bufs=1, space="DRAM"))
        a2a_in = dram.tile([1024, 512], BF16)
        a2a_out = dram.tile([1024, 512], BF16)
        a2a12_in = dram.tile([1024, 384], BF16)
        a2a12_out = dram.tile([1024, 384], BF16)

        xtp = stk.enter_context(tc.tile_pool(name="xtp", bufs=1))
        esp = stk.enter_context(tc.tile_pool(name="esp", bufs=1))
        otsb = stk.enter_context(tc.tile_pool(name="otsb", bufs=1))
        smallp = stk.enter_context(tc.tile_pool(name="smallp", bufs=1))
        qkvps = stk.enter_context(tc.tile_pool(name="qkvps", bufs=1, space="PSUM"))
        spps = stk.enter_context(tc.tile_pool(name="spps", bufs=1, space="PSUM"))
        otps = stk.enter_context(tc.tile_pool(name="otps", bufs=1, space="PSUM"))

        def load_xt(b, n):
            """Prefetch the window's x^T chunk tiles (depth-2 ahead so no xt
            DMA is ever issued after a collective instruction — DMAs issued
            after a collective wait for its completion)."""
            xts_t = []
            for k in range(8):
                t = xtp.tile([128, G], BF16, tag=f"xt{k}", bufs=3,
                             name=f"xt_t{k}")
                nc.sync.dma_start(t[:], xts[b][128 * k:128 * (k + 1),
                                               G * n:G * (n + 1)])
                xts_t.append(t)
            return xts_t

        def p1_pieces(b, n, xts_t):
            """Closures for the PE-side pieces (2 q-groups, 2 k-groups, 2
            V^T-groups) interleaved between score-phase issues of the
            previous window. V is produced directly in key-major layout by
            flipping the matmul operands (lhsT=x chunk, rhs=w_v), so no
            PE transposes are needed."""

            def qk_piece(s_, nm):
                def emit():
                    ps = qkvps.tile([128, 512], F32, tag="ps", bufs=2,
                                    name="ps")
                    for k in range(8):
                        nc.tensor.matmul(ps[:],
                                         w_sb[nm][:, 128 * k:128 * (k + 1)],
                                         xts_t[k][:, 512 * s_:512 * (s_ + 1)],
                                         start=(k == 0), stop=(k == 7))
                    t_ = QT[b] if nm == "q" else KTb[b]
                    nc.vector.tensor_copy(
                        t_[:, G * n + 512 * s_:G * n + 512 * (s_ + 1)], ps[:])
                return emit

            def vt_piece(s_):
                def emit():
                    ps = qkvps.tile([128, 512], F32, tag="ps", bufs=2,
                                    name="psv")
                    for tl in range(4):
                        t = 4 * s_ + tl
                        for k in range(8):
                            nc.tensor.matmul(
                                ps[:, 128 * tl:128 * (tl + 1)],
                                xts_t[k][:, 128 * t:128 * (t + 1)],
                                w_sb["v"][:, 128 * k:128 * (k + 1)],
                                start=(k == 0), stop=(k == 7))
                    for tl in range(4):
                        t = 4 * s_ + tl
                        vdst = Vaug[b][:, 130 * (8 * n + t):
                                       130 * (8 * n + t + 1)]
                        nc.vector.tensor_copy(
                            vdst.rearrange("p (h c) -> p h c", c=65)
                            [:, :, 0:64],
                            ps[:, 128 * tl:128 * (tl + 1)]
                            .rearrange("p (h c) -> p h c", c=64))
                return emit

            pieces = []
            for s_ in range(2):
                pieces.append(qk_piece(s_, "q"))
                pieces.append(qk_piece(s_, "k"))
                pieces.append(vt_piece(s_))
            return pieces

        KT_GROUPS = [(0,), (1,), (2,), (3,), (4, 5), (6, 7)]

        def phase1(b, n, pieces=()):
            """Scores + exp for window (b, n), interleaving the next
            window's P1 pieces between key-tile issues to keep the PE fed
            while the Activation engine works through the exponentials.
            Small key-tiles (kt 4+5, 6+7) share one score tile and one
            exp instruction to amortize the Activation per-op overhead."""
            pieces = list(pieces)
            ES = {}
            for gi, kts in enumerate(KT_GROUPS):
                for hh in range(2):
                    hs = 64 * hh
                    sp = spps.tile([128, 1024], F32, tag="sp", bufs=2,
                                   name="sp")
                    off = 0
                    offs = []
                    for kt in kts:
                        nq = G - 128 * kt
                        base = G * n + 128 * kt
                        lhsT = KTb[b][hs:hs + 64, base:base + 128]
                        for c0 in range(0, nq, 512):
                            c1 = min(c0 + 512, nq)
                            nc.tensor.matmul(
                                sp[:, off + c0:off + c1], lhsT,
                                QT[b][hs:hs + 64, base + c0:base + c1],
                                start=True, stop=True)
                        offs.append(off)
                        off += nq
                    es = esp.tile([128, off], BF16, tag=f"esg{gi}{hh}",
                                  bufs=2, name=f"esg{gi}{hh}")
                    nc.scalar.activation(es[:], sp[:, 0:off],
                                         mybir.ActivationFunctionType.Exp)
                    for kt, o in zip(kts, offs):
                        nc.vector.tensor_mul(es[:, o:o + 128],
                                             es[:, o:o + 128], tri_sb[:])
                        ES[(kt, hh)] = (es, o)
                if pieces:
                    pieces.pop(0)()
                    if gi >= 3 and pieces:
                        pieces.pop(0)()
            while pieces:
                pieces.pop(0)()
            return ES

        def phase2(b, n, ES):
            ots_w = otsb.tile([128, 1024], BF16, tag="ots", bufs=4,
                              name="ots_w")
            for hh in range(2):
                OT = [otps.tile([128, 512], F32, tag=f"ot{hf}", bufs=1,
                                name=f"OT{hf}") for hf in range(2)]
                for qt in range(KT):
                    o = OT[qt // 4]
                    for kt in range(qt + 1):
                        va = Vaug[b][:, 130 * (KT * n + kt) + 65 * hh:
                                     130 * (KT * n + kt) + 65 * hh + 65]
                        est, eo = ES[(kt, hh)]
                        nc.tensor.matmul(
                            o[:, 65 * (qt % 4):65 * (qt % 4) + 65],
                            est[:, eo + 128 * (qt - kt):
                                eo + 128 * (qt - kt) + 128],
                            va, start=(kt == 0), stop=(kt == qt),
                            skip_group_check=True)
                for hf in range(2):
                    o = OT[hf]
                    den = smallp.tile([128, 4], F32, tag=f"dn{hh}{hf}",
                                      bufs=2, name="den")
                    o_g = o[:, 0:260].rearrange("p (t c) -> p t c", c=65)
                    nc.vector.tensor_mul(
                        den[:].rearrange("p (t c) -> p t c", c=1),
                        o_g[:, :, 64:65],
                        vinv_sb[:, TOFF[b] + 8 * n + 4 * hf:
                                TOFF[b] + 8 * n + 4 * hf + 4]
                        .rearrange("p (t c) -> p t c", c=1))
                    sc = smallp.tile([128, 4], F32, tag=f"sc{hh}{hf}",
                                     bufs=2, name="scl")
                    nc.vector.reciprocal_approx_fast(sc[:], den[:])
                    for qp in range(4):
                        qt = 4 * hf + qp
                        nc.vector.tensor_scalar_mul(
                            ots_w[:, 128 * qt + 64 * hh:
                                  128 * qt + 64 * hh + 64],
                            o[:, 65 * qp:65 * qp + 64],
                            sc[:, qp:qp + 1])
            # q-major staging: ship [128 q, 128 feat] pieces as-is; the
            # receiver transposes after the exchange (PE). No DmaTranspose
            # anywhere -> no scheduler serialization against collectives.
            if b == 0:
                for t in range(2):
                    j = 2 * n + t
                    nc.sync.dma_start(a2a_in[128 * j:128 * (j + 1), :],
                                      ots_w[:, 512 * t:512 * (t + 1)])
            elif b == 1:
                for qt in range(KT):
                    j = 4 * n + qt // 2
                    c0 = 128 * (qt % 2)
                    nc.sync.dma_start(
                        a2a12_in[128 * j:128 * (j + 1), c0:c0 + 128],
                        ots_w[:, 128 * qt:128 * (qt + 1)])
            else:
                for qt in range(KT):
                    nc.sync.dma_start(
                        a2a12_in[128 * qt:128 * (qt + 1), 256:384],
                        ots_w[:, 128 * qt:128 * (qt + 1)])

        # ---- software-pipelined steps ----------------------------------------
        sprep = stk.enter_context(tc.tile_pool(name="sprep", bufs=1))
        S = sprep.tile([128, 4096], BF16, name="S")
        nc.vector.memset(S[:], 0.0)
        sprep_tiles = []

        for half in range(2):
            for name, ap in (("q", wq), ("k", wk), ("v", wv)):
                nc.sync.dma_start(
                    w_sb[name][:, 512 * half:512 * (half + 1)]
                    .rearrange("p (k j) -> p k j", k=4),
                    ap[512 * half:512 * (half + 1), :]
                    .rearrange("(k p) j -> p k j", p=128))
        xt_pre = [load_xt(*STEPS[0])]
        boot = p1_pieces(*STEPS[0], xt_pre[0])
        nc.sync.dma_start(ident_sb[:], ident[:])
        nc.sync.dma_start(tri_sb[:], trimask[:])
        nc.sync.dma_start(vinv_sb[:], vinv[:])
        xt_pre.append(load_xt(*STEPS[1]))
        for p in boot:
            p()
        prev = None
        for i, (b, n) in enumerate(STEPS):
            if i + 2 < len(STEPS):
                xt_pre.append(load_xt(*STEPS[i + 2]))
            nxt = (p1_pieces(*STEPS[i + 1], xt_pre[i + 1])
                   if i + 1 < len(STEPS) else [])
            es_now = phase1(b, n, nxt)
            if i == 1:
                nc.sync.dma_start(wp_all[:].rearrange("p (k j) -> p k j", k=8),
                                  wproj.rearrange("(k p) j -> p k j", p=128))
            if prev is not None:
                pb, pn, pes = prev
                phase2(pb, pn, pes)
                if (pb, pn) == (0, 0):
                    # textually late: every DMA issued after a collective
                    # waits for it; here only naturally-late stores follow.
                    # Its start is dep-driven (b1/b2 staging), so it still
                    # overlaps the branch-0 steps.
                    nc.gpsimd.collective_compute(
                        "AllToAll", mybir.AluOpType.bypass,
                        replica_groups=[list(range(N_CORES))],
                        ins=[a2a12_in.opt()], outs=[a2a12_out.opt()])
                if (pb, pn) == (0, 1):
                    # a2a12 results: loads issued here (blocked only until
                    # the collective completes, harmless on the DMA queue)
                    T1 = sprep.tile([128, 8 * 256], BF16, name="T1")
                    nc.sync.dma_start(T1[:].rearrange("p (k j) -> p k j", k=8),
                                      a2a12_out[:, 0:256]
                                      .rearrange("(k p) j -> p k j", p=128))
                    T2 = sprep.tile([128, 8 * 128], BF16, name="T2")
                    nc.sync.dma_start(T2[:].rearrange("p (k j) -> p k j", k=8),
                                      a2a12_out[:, 256:384]
                                      .rearrange("(k p) j -> p k j", p=128))
                    sprep_tiles.extend([T1, T2])
                if (pb, pn) == (0, 2):
                    # prebuild the sparse branch sum S one step later so the
                    # PE never queues behind the collective-gated loads:
                    # transpose each received [128 q, 128 f] piece on the PE
                    # and scatter straight from PSUM into S.
                    T1, T2 = sprep_tiles

                    def tr_slot_pool():
                        state = {}
                        def get():
                            i = state.get("i", 0)
                            if i % 4 == 0:
                                state["t"] = qkvps.tile(
                                    [128, 512], F32, tag="ps", bufs=2,
                                    name="pst")
                            state["i"] = i + 1
                            sl = state["t"][:, 64 * (i % 4):64 * (i % 4) + 64]
                            return sl.bitcast(BF16)
                        return get
                    slot = tr_slot_pool()
                    for jj in range(8):
                        i2, i4 = jj // 4, jj // 2
                        for s_ in range(2):
                            ptr = slot()
                            nc.tensor.transpose(
                                ptr,
                                T1[:, 256 * jj + 128 * s_:
                                   256 * jj + 128 * (s_ + 1)],
                                ident_sb[:])
                            sl = S[:, 512 * jj + 256 * s_:
                                   512 * jj + 256 * (s_ + 1)]
                            nc.vector.tensor_copy(
                                sl.rearrange("p (t c) -> p t c", c=2)
                                [:, :, i2:i2 + 1],
                                ptr.rearrange("p (t c) -> p t c", c=1))
                        ptr = slot()
                        nc.tensor.transpose(ptr,
                                            T2[:, 128 * jj:128 * (jj + 1)],
                                            ident_sb[:])
                        s4 = S[:, 512 * jj:512 * (jj + 1)] \
                            .rearrange("p (t c) -> p t c", c=4)
                        nc.vector.tensor_add(
                            s4[:, :, i4:i4 + 1], s4[:, :, i4:i4 + 1],
                            ptr.rearrange("p (t c) -> p t c", c=1))
            prev = (b, n, es_now)
        phase2(*prev)
        nc.gpsimd.collective_compute(
            "AllToAll", mybir.AluOpType.bypass,
            replica_groups=[list(range(N_CORES))],
            ins=[a2a_in.opt()], outs=[a2a_out.opt()])

        # ---- P4: dense slice + prebuilt sparse sum ----------------------------
        with (tc.tile_pool(name="ptp", bufs=1) as ptp,
              tc.tile_pool(name="ocp", bufs=1) as ocp):
            PTq = ptp.tile([128, 8 * 512], BF16, name="PTq")
            for jj in range(8):
                nc.sync.dma_start(PTq[:, 512 * jj:512 * (jj + 1)],
                                  a2a_out[128 * jj:128 * (jj + 1), :])
            PT = ptp.tile([128, 8 * 512], BF16, name="PT")
            # s_-major so P5's m=s_ accumulations can start after one batch;
            # the branch-sum S is fused into the PSUM->PT move.
            for s_ in range(4):
                for g in range(2):
                    pst2 = qkvps.tile([128, 512], F32, tag="ps", bufs=2,
                                      name="pst2")
                    for q_ in range(4):
                        jj = 4 * g + q_
                        ptr = pst2[:, 64 * q_:64 * (q_ + 1)].bitcast(BF16)
                        nc.tensor.transpose(
                            ptr, PTq[:, 512 * jj + 128 * s_:
                                     512 * jj + 128 * (s_ + 1)], ident_sb[:])
                    # one grouped add for the 4 transposed pieces
                    pt4 = PT[:].rearrange("p (t s c) -> p t s c", s=4, c=128)
                    s4_ = S[:].rearrange("p (t s c) -> p t s c", s=4, c=128)
                    nc.vector.tensor_add(
                        pt4[:, 4 * g:4 * g + 4, s_:s_ + 1, :],
                        pst2[:, 0:256].bitcast(BF16)
                        .rearrange("p (t s c) -> p t s c", s=1, c=128),
                        s4_[:, 4 * g:4 * g + 4, s_:s_ + 1, :])

            # ---- P5: projection (folded output store) -------------------------
            oc_all = ocp.tile([128, 4096], F32, name="oc_all")
            for m in range(4):
                for nb in range(2):
                    pp = qkvps.tile([128, 512], F32, tag="ps", bufs=2,
                                    name="pp")
                    for jj in range(8):
                        nc.tensor.matmul(
                            pp[:], PT[:, 512 * jj + 128 * m:
                                       512 * jj + 128 * (m + 1)],
                            wp_all[:, E * jj + 512 * nb:E * jj + 512 * (nb + 1)],
                            start=(jj == 0), stop=(jj == 7))
                    nc.vector.tensor_copy(
                        oc_all[:, 1024 * m + 512 * nb:1024 * m + 512 * (nb + 1)],
                        pp[:])
                nc.sync.dma_start(out[128 * m:128 * (m + 1), :],
                                  oc_all[:, 1024 * m:1024 * (m + 1)])
    nc.compile()
    return nc


_NC_CACHE = None


def _get_nc():
    global _NC_CACHE
    if _NC_CACHE is None:
        _NC_CACHE = build_nc()
    return _NC_CACHE


def _host_inputs(x, w_qkv, w_proj):
    xT = np.ascontiguousarray(x[0].T).astype(np.float32)      # (E, L)
    ident = np.eye(128, dtype=np.float32)
    import ml_dtypes
    f = np.arange(128)
    trimask = np.where(f[None, :] >= f[:, None], 1.0, 0.0).astype(ml_dtypes.bfloat16)
    in_maps = []
    for c in range(N_CORES):
        h = 2 * c
        vps = []
        for b, r in enumerate(RATIOS):
            i = h // (16 // r)
            cs = r * np.arange(L // r) + i
            V = 1 + (cs % 2 == h // 8).astype(np.int32) \
                  + (cs % 4 == h // 4).astype(np.int32)
            vps.append(V.astype(np.float32))
        vinv2 = np.concatenate(vps).reshape(56, 128).T        # (128, 56)
        i2, i4 = c // 4, c // 2
        m = {
            "xt0": xT,
            "xt1": np.ascontiguousarray(xT[:, i2::2]),
            "xt2": np.ascontiguousarray(xT[:, i4::4]),
            "wq": np.ascontiguousarray(w_qkv[:, 128 * c:128 * (c + 1)]) / 8.0,
            "wk": np.ascontiguousarray(w_qkv[:, E + 128 * c:E + 128 * (c + 1)]),
            "wv": np.ascontiguousarray(w_qkv[:, 2 * E + 128 * c:2 * E + 128 * (c + 1)]),
            "wproj": np.ascontiguousarray(w_proj),
            "ident": ident,
            "trimask": trimask,
            "vinv": np.ascontiguousarray(vinv2),
        }
        bf = ("trimask", "ident", "xt0", "xt1", "xt2", "wq", "wk", "wv", "wproj")
        in_maps.append({k: np.ascontiguousarray(
                            v if k == "trimask" else
                            np.asarray(v, np.float32).astype(ml_dtypes.bfloat16))
                        if k in bf
                        else np.ascontiguousarray(v, dtype=np.float32)
                        for k, v in m.items()})
    return in_maps


def kernel(x, w_qkv, w_proj, _trace=False):
    x = np.asarray(x, np.float32)
    w_qkv = np.asarray(w_qkv, np.float32)
    w_proj = np.asarray(w_proj, np.float32)
    nc = _get_nc()
    in_maps = _host_inputs(x, w_qkv, w_proj)
    res = run_bass_kernel_spmd(nc, in_maps, core_ids=list(range(N_CORES)),
                               trace=_trace)
    full = np.empty((L, E), np.float32)
    for c in range(N_CORES):
        full[512 * c:512 * (c + 1)] = res.results[c]["out"]
    out = full.reshape(1, L, E)
    if _trace:
        return out, res
    return out
